# revision 7
# baseline (speedup 1.0000x reference)
"""Conditional Instance Norm (CIN) kernel for Trainium2, data-parallel over batch.

Reference semantics (per batch sample b, channel c):
    gamma_mix = style_weights @ gammas          # [B, C]
    beta_mix  = style_weights @ betas           # [B, C]
    y[b,c]    = gamma_mix[b,c] * (x[b,c] - mean) * rsqrt(var + eps) + beta_mix[b,c]
with mean/var over the spatial dims of x[b,c] (biased var).

Strategy: one batch sample per NeuronCore (B=8 samples, 8 cores).  Per core,
x is [C=256, HW=65536] fp32 = 64 MiB.  Channels are processed in tiles of
G=32 channels; each channel's HW elements are laid out over Q=128/G
partitions, so a tile is a dense [128, F=HW/Q] SBUF block read from HBM
exactly once and written exactly once: 128 MiB of HBM traffic per core,
the memory-regime floor.

Per tile:
  DVE reduce_sum               -> per-partition sums   [128,1]
  ACT Square w/ accum_out      -> per-partition sumsq  [128,1] (square result
                                  dumped to a bf16 scratch that's never read)
  PE matmul w/ 1/HW selector   -> per-channel (mean, E[x^2])  [G,2]
  tiny DVE/ACT ops             -> scale = gamma*rsqrt(var+eps),
                                  bias  = beta - mean*scale   [G,2]
  PE matmul w/ 0/1 expander    -> per-partition (scale, bias) [128,2]
  ACT Identity (scale,bias AP) -> y = scale*x + bias, in place

PE Matmult instructions only tolerate a single sync-wait, so every matmul
operand is funneled through a DVE-produced tile (one wait, one engine):
the constants arrive in a single packed DMA + one DVE copy, and the
two-engine (sum, sumsq) pair goes through a tiny DVE copy.
"""

import sys

for _p in ("/opt/trn_rl_repo",):
    if _p not in sys.path:
        sys.path.insert(0, _p)

from contextlib import ExitStack

import numpy as np

import concourse.bacc as bacc
import concourse.bass as bass
import concourse.tile as tile
from concourse import mybir
from concourse.bass_utils import run_bass_kernel_spmd

EPS = 1e-5

# Full problem dims (hardcoded per harness contract).
B, C, H, W = 8, 256, 256, 256
S = 16
HW = H * W
N_CORES = 8
P = 128  # SBUF partitions

AF = mybir.ActivationFunctionType
f32 = mybir.dt.float32
bf16 = mybir.dt.bfloat16


def _const_layout(C_, S_, G):
    """Column offsets of the packed constants tensor: g4 | e4 | gammas | betas | sw."""
    o_g4 = 0
    o_e4 = o_g4 + G
    o_gam = o_e4 + P
    o_bet = o_gam + C_
    o_sw = o_bet + C_
    ncols = o_sw + 1
    return o_g4, o_e4, o_gam, o_bet, o_sw, ncols


def build_cin_program(
    C_=C,
    HW_=HW,
    S_=S,
    G=32,  # channels per tile
    xt_bufs=2,
    apply_engine="act",  # "act" or "dve"
):
    """Trace the per-core CIN program.  Returns the Bass module."""
    Q = P // G  # partitions per channel
    F = HW_ // Q  # free elems per partition
    NT = C_ // G  # number of tiles
    assert P % G == 0 and HW_ % Q == 0 and C_ % G == 0

    o_g4, o_e4, o_gam, o_bet, o_sw, NCOLS = _const_layout(C_, S_, G)

    nc = bacc.Bacc(trn_type="TRN2")

    x_d = nc.dram_tensor("x", [C_ * Q, F], f32, kind="ExternalInput")
    consts_d = nc.dram_tensor("consts", [P, NCOLS], f32, kind="ExternalInput")
    y_d = nc.dram_tensor("y", [C_ * Q, F], f32, kind="ExternalOutput")

    with tile.TileContext(nc) as tc, ExitStack() as ctx:
        xpool = ctx.enter_context(tc.tile_pool(name="xt", bufs=xt_bufs))
        sqpool = ctx.enter_context(tc.tile_pool(name="sq", bufs=1))
        ppool = ctx.enter_context(tc.tile_pool(name="part", bufs=4))
        stpool = ctx.enter_context(tc.tile_pool(name="st", bufs=4))
        sbpool = ctx.enter_context(tc.tile_pool(name="sb", bufs=4))
        singles = ctx.enter_context(tc.tile_pool(name="singles", bufs=1))
        ch_ps = ctx.enter_context(tc.tile_pool(name="chps", bufs=2, space="PSUM"))
        bc_ps = ctx.enter_context(tc.tile_pool(name="bcps", bufs=2, space="PSUM"))
        gb_psp = ctx.enter_context(tc.tile_pool(name="gbps", bufs=1, space="PSUM"))

        # ---- constants: one DMA + one DVE funnel copy ----
        consts_sb = singles.tile([P, NCOLS], f32)
        nc.gpsimd.dma_start(out=consts_sb[:], in_=consts_d[:])
        consts_f = singles.tile([P, NCOLS], f32)
        nc.vector.tensor_copy(consts_f[:], consts_sb[:])

        g4_f = consts_f[:, o_g4 : o_g4 + G]  # [128, G] selector, 1/HW entries
        e4_f = consts_f[0:G, o_e4 : o_e4 + P]  # [G, 128] expander, 0/1 entries
        sw_f = consts_f[0:S_, o_sw : o_sw + 1]  # [S, 1]

        eps_sb = singles.tile([G, 1], f32)
        nc.vector.memset(eps_sb[:], EPS)

        # gb_all[:, t, 0] = gamma_mix for tile t's channels, [:, t, 1] = beta_mix
        gb_ps = gb_psp.tile([G, NT, 2], f32)
        gb_all = singles.tile([G, NT, 2], f32)
        for t in range(NT):
            gam_t = consts_f[0:S_, o_gam + G * t : o_gam + G * (t + 1)]
            bet_t = consts_f[0:S_, o_bet + G * t : o_bet + G * (t + 1)]
            nc.tensor.matmul(gb_ps[:, t, 0:1], gam_t, sw_f, start=True, stop=True)
            nc.tensor.matmul(gb_ps[:, t, 1:2], bet_t, sw_f, start=True, stop=True)
        nc.vector.tensor_copy(gb_all[:], gb_ps[:])

        # ---- main loop over channel tiles ----
        for t in range(NT):
            xt = xpool.tile([P, F], f32)
            nc.sync.dma_start(out=xt[:], in_=x_d[P * t : P * (t + 1), :])

            # per-partition sum and sum-of-squares
            part = ppool.tile([P, 2], f32)
            nc.vector.reduce_sum(part[:, 0:1], xt[:], axis=mybir.AxisListType.X)
            sq = sqpool.tile([P, F], bf16)
            nc.scalar.activation(
                out=sq[:], in_=xt[:], func=AF.Square, accum_out=part[:, 1:2]
            )
            # funnel both stats through DVE so the PE matmul needs one wait
            part2 = ppool.tile([P, 2], f32, tag="part2")
            nc.vector.tensor_copy(part2[:], part[:])

            # fold Q partitions -> per-channel (mean, E[x^2])
            ch = ch_ps.tile([G, 2], f32)
            nc.tensor.matmul(ch[:], g4_f, part2[:], start=True, stop=True)

            # st columns: 0=mean 1=exsq 2=tmp 3=var 4=scale 5=bias 6=std 7=rstd
            st = stpool.tile([G, 8], f32)
            nc.vector.tensor_copy(st[:, 0:2], ch[:])
            nc.vector.tensor_mul(st[:, 2:3], st[:, 0:1], st[:, 0:1])
            nc.vector.tensor_sub(st[:, 3:4], st[:, 1:2], st[:, 2:3])
            nc.scalar.activation(
                out=st[:, 6:7], in_=st[:, 3:4], func=AF.Sqrt, bias=eps_sb[:]
            )
            nc.vector.reciprocal(st[:, 7:8], st[:, 6:7])
            nc.vector.tensor_mul(st[:, 4:5], st[:, 7:8], gb_all[:, t, 0:1])
            nc.vector.tensor_mul(st[:, 2:3], st[:, 0:1], st[:, 4:5])
            nc.vector.tensor_sub(st[:, 5:6], gb_all[:, t, 1:2], st[:, 2:3])

            # broadcast per-channel (scale, bias) back to the Q partitions each
            bc = bc_ps.tile([P, 2], f32)
            nc.tensor.matmul(bc[:], e4_f, st[:, 4:6], start=True, stop=True)
            sb2 = sbpool.tile([P, 2], f32)
            nc.vector.tensor_copy(sb2[:], bc[:])

            # y = scale * x + bias, in place
            if apply_engine == "act":
                nc.scalar.activation(
                    out=xt[:],
                    in_=xt[:],
                    func=AF.Identity,
                    bias=sb2[:, 1:2],
                    scale=sb2[:, 0:1],
                )
            else:
                nc.vector.tensor_scalar(
                    out=xt[:],
                    in0=xt[:],
                    scalar1=sb2[:, 0:1],
                    scalar2=sb2[:, 1:2],
                    op0=mybir.AluOpType.mult,
                    op1=mybir.AluOpType.add,
                )

            nc.gpsimd.dma_start(out=y_d[P * t : P * (t + 1), :], in_=xt[:])

    nc.compile()
    return nc


def make_consts(C_=C, HW_=HW, S_=S, G=32, gammas=None, betas=None, sw=None):
    """Host-side packed constants tensor [128, NCOLS]."""
    Q = P // G
    o_g4, o_e4, o_gam, o_bet, o_sw, NCOLS = _const_layout(C_, S_, G)
    consts = np.zeros((P, NCOLS), np.float32)
    consts[np.arange(P), o_g4 + np.arange(P) // Q] = 1.0 / HW_
    consts[np.arange(P) // Q, o_e4 + np.arange(P)] = 1.0
    consts[0:S_, o_gam : o_gam + C_] = gammas
    consts[0:S_, o_bet : o_bet + C_] = betas
    consts[0:S_, o_sw] = sw
    return consts


_CACHE = {}


def _get_nc():
    if "nc" not in _CACHE:
        _CACHE["nc"] = build_cin_program()
    return _CACHE["nc"]


def kernel(x, style_weights, gammas, betas, _trace=False):
    x = np.ascontiguousarray(np.asarray(x, dtype=np.float32))
    style_weights = np.ascontiguousarray(np.asarray(style_weights, dtype=np.float32))
    gammas = np.ascontiguousarray(np.asarray(gammas, dtype=np.float32))
    betas = np.ascontiguousarray(np.asarray(betas, dtype=np.float32))

    G = 32
    Q = P // G
    F = HW // Q
    nc = _get_nc()

    xr = x.reshape(B, C * Q, F)
    in_maps = [
        {
            "x": xr[i],
            "consts": make_consts(C, HW, S, G, gammas, betas, style_weights[i]),
        }
        for i in range(N_CORES)
    ]
    res = run_bass_kernel_spmd(
        nc, in_maps, core_ids=list(range(N_CORES)), trace=_trace
    )
    y = np.stack(
        [res.results[i]["y"].reshape(C, H, W) for i in range(N_CORES)], axis=0
    )
    if _trace:
        return y, res
    return y


# revision 9
# speedup vs baseline: 223.9737x; 223.9737x over previous
"""Conditional Instance Norm (CIN) kernel for Trainium2, data-parallel over batch.

Reference semantics (per batch sample b, channel c):
    gamma_mix = style_weights @ gammas          # [B, C]
    beta_mix  = style_weights @ betas           # [B, C]
    y[b,c]    = gamma_mix[b,c] * (x[b,c] - mean) * rsqrt(var + eps) + beta_mix[b,c]
with mean/var over the spatial dims of x[b,c] (biased var).

Strategy: one batch sample per NeuronCore (B=8 samples, 8 cores).  Per core,
x is [C=256, HW=65536] fp32 = 64 MiB.  Channels are processed in tiles of
G=32 channels; each channel's HW elements are laid out over Q=128/G
partitions, so a tile is a dense [128, F=HW/Q] SBUF block read from HBM
exactly once and written exactly once: 128 MiB of HBM traffic per core,
the memory-regime floor.

Per tile:
  DVE reduce_sum               -> per-partition sums   [128,1]
  ACT Square w/ accum_out      -> per-partition sumsq  [128,1] (square result
                                  dumped to a bf16 scratch that's never read)
  PE matmul w/ 1/HW selector   -> per-channel (mean, E[x^2])  [G,2]
  tiny DVE/ACT ops             -> scale = gamma*rsqrt(var+eps),
                                  bias  = beta - mean*scale   [G,2]
  PE matmul w/ 0/1 expander    -> per-partition (scale, bias) [128,2]
  ACT Identity (scale,bias AP) -> y = scale*x + bias, in place

PE Matmult instructions only tolerate a single sync-wait, so every matmul
operand is funneled through a DVE-produced tile (one wait, one engine):
the constants arrive in a single packed DMA + one DVE copy, and the
two-engine (sum, sumsq) pair goes through a tiny DVE copy.
"""

import sys

for _p in ("/opt/trn_rl_repo",):
    if _p not in sys.path:
        sys.path.insert(0, _p)

from contextlib import ExitStack

import numpy as np

import concourse.bacc as bacc
import concourse.bass as bass
import concourse.tile as tile
from concourse import mybir
from concourse.bass_utils import run_bass_kernel_spmd

EPS = 1e-5

# Full problem dims (hardcoded per harness contract).
B, C, H, W = 8, 256, 256, 256
S = 16
HW = H * W
N_CORES = 8
P = 128  # SBUF partitions

AF = mybir.ActivationFunctionType
f32 = mybir.dt.float32
bf16 = mybir.dt.bfloat16


def _const_layout(C_, S_, G):
    """Column offsets of the packed constants tensor: g4 | e4 | gammas | betas | sw."""
    o_g4 = 0
    o_e4 = o_g4 + G
    o_gam = o_e4 + P
    o_bet = o_gam + C_
    o_sw = o_bet + C_
    ncols = o_sw + 1
    return o_g4, o_e4, o_gam, o_bet, o_sw, ncols


def build_cin_program(
    C_=C,
    HW_=HW,
    S_=S,
    G=32,  # channels per tile
    xt_bufs=2,
    apply_engine="act",  # "act" or "dve"
    reps=1,  # repeat the main loop (for slope-based benchmarking)
):
    """Trace the per-core CIN program.  Returns the Bass module."""
    Q = P // G  # partitions per channel
    F = HW_ // Q  # free elems per partition
    NT = C_ // G  # number of tiles
    assert P % G == 0 and HW_ % Q == 0 and C_ % G == 0

    o_g4, o_e4, o_gam, o_bet, o_sw, NCOLS = _const_layout(C_, S_, G)

    nc = bacc.Bacc(trn_type="TRN2")

    x_d = nc.dram_tensor("x", [C_ * Q, F], f32, kind="ExternalInput")
    consts_d = nc.dram_tensor("consts", [P, NCOLS], f32, kind="ExternalInput")
    y_d = nc.dram_tensor("y", [C_ * Q, F], f32, kind="ExternalOutput")

    with tile.TileContext(nc) as tc, ExitStack() as ctx:
        xpool = ctx.enter_context(tc.tile_pool(name="xt", bufs=xt_bufs))
        sqpool = ctx.enter_context(tc.tile_pool(name="sq", bufs=1))
        ppool = ctx.enter_context(tc.tile_pool(name="part", bufs=4))
        stpool = ctx.enter_context(tc.tile_pool(name="st", bufs=4))
        sbpool = ctx.enter_context(tc.tile_pool(name="sb", bufs=4))
        singles = ctx.enter_context(tc.tile_pool(name="singles", bufs=1))
        ch_ps = ctx.enter_context(tc.tile_pool(name="chps", bufs=2, space="PSUM"))
        bc_ps = ctx.enter_context(tc.tile_pool(name="bcps", bufs=2, space="PSUM"))
        gb_psp = ctx.enter_context(tc.tile_pool(name="gbps", bufs=1, space="PSUM"))

        # ---- constants: one DMA + one DVE funnel copy ----
        consts_sb = singles.tile([P, NCOLS], f32)
        nc.gpsimd.dma_start(out=consts_sb[:], in_=consts_d[:])
        consts_f = singles.tile([P, NCOLS], f32)
        nc.vector.tensor_copy(consts_f[:], consts_sb[:])

        g4_f = consts_f[:, o_g4 : o_g4 + G]  # [128, G] selector, 1/HW entries
        e4_f = consts_f[0:G, o_e4 : o_e4 + P]  # [G, 128] expander, 0/1 entries
        sw_f = consts_f[0:S_, o_sw : o_sw + 1]  # [S, 1]

        eps_sb = singles.tile([G, 1], f32)
        nc.vector.memset(eps_sb[:], EPS)

        # gb_all[:, t, 0] = gamma_mix for tile t's channels, [:, t, 1] = beta_mix
        gb_ps = gb_psp.tile([G, NT, 2], f32)
        gb_all = singles.tile([G, NT, 2], f32)
        for t in range(NT):
            gam_t = consts_f[0:S_, o_gam + G * t : o_gam + G * (t + 1)]
            bet_t = consts_f[0:S_, o_bet + G * t : o_bet + G * (t + 1)]
            nc.tensor.matmul(gb_ps[:, t, 0:1], gam_t, sw_f, start=True, stop=True)
            nc.tensor.matmul(gb_ps[:, t, 1:2], bet_t, sw_f, start=True, stop=True)
        nc.vector.tensor_copy(gb_all[:], gb_ps[:])

        # ---- main loop over channel tiles ----
        for t in [t for _ in range(reps) for t in range(NT)]:
            xt = xpool.tile([P, F], f32)
            nc.sync.dma_start(out=xt[:], in_=x_d[P * t : P * (t + 1), :])

            # per-partition sum and sum-of-squares
            part = ppool.tile([P, 2], f32)
            nc.vector.reduce_sum(part[:, 0:1], xt[:], axis=mybir.AxisListType.X)
            sq = sqpool.tile([P, F], bf16)
            nc.scalar.activation(
                out=sq[:], in_=xt[:], func=AF.Square, accum_out=part[:, 1:2]
            )
            # funnel both stats through DVE so the PE matmul needs one wait
            part2 = ppool.tile([P, 2], f32, tag="part2")
            nc.vector.tensor_copy(part2[:], part[:])

            # fold Q partitions -> per-channel (mean, E[x^2])
            ch = ch_ps.tile([G, 2], f32)
            nc.tensor.matmul(ch[:], g4_f, part2[:], start=True, stop=True)

            # st columns: 0=mean 1=exsq 2=tmp 3=var 4=scale 5=bias 6=std 7=rstd
            st = stpool.tile([G, 8], f32)
            nc.vector.tensor_copy(st[:, 0:2], ch[:])
            nc.vector.tensor_mul(st[:, 2:3], st[:, 0:1], st[:, 0:1])
            nc.vector.tensor_sub(st[:, 3:4], st[:, 1:2], st[:, 2:3])
            nc.scalar.activation(
                out=st[:, 6:7], in_=st[:, 3:4], func=AF.Sqrt, bias=eps_sb[:]
            )
            nc.vector.reciprocal(st[:, 7:8], st[:, 6:7])
            nc.vector.tensor_mul(st[:, 4:5], st[:, 7:8], gb_all[:, t, 0:1])
            nc.vector.tensor_mul(st[:, 2:3], st[:, 0:1], st[:, 4:5])
            nc.vector.tensor_sub(st[:, 5:6], gb_all[:, t, 1:2], st[:, 2:3])

            # broadcast per-channel (scale, bias) back to the Q partitions each
            bc = bc_ps.tile([P, 2], f32)
            nc.tensor.matmul(bc[:], e4_f, st[:, 4:6], start=True, stop=True)
            sb2 = sbpool.tile([P, 2], f32)
            nc.vector.tensor_copy(sb2[:], bc[:])

            # y = scale * x + bias, in place
            if apply_engine == "act":
                nc.scalar.activation(
                    out=xt[:],
                    in_=xt[:],
                    func=AF.Identity,
                    bias=sb2[:, 1:2],
                    scale=sb2[:, 0:1],
                )
            else:
                nc.vector.tensor_scalar(
                    out=xt[:],
                    in0=xt[:],
                    scalar1=sb2[:, 0:1],
                    scalar2=sb2[:, 1:2],
                    op0=mybir.AluOpType.mult,
                    op1=mybir.AluOpType.add,
                )

            nc.gpsimd.dma_start(out=y_d[P * t : P * (t + 1), :], in_=xt[:])

    nc.compile()
    return nc


def make_consts(C_=C, HW_=HW, S_=S, G=32, gammas=None, betas=None, sw=None):
    """Host-side packed constants tensor [128, NCOLS]."""
    Q = P // G
    o_g4, o_e4, o_gam, o_bet, o_sw, NCOLS = _const_layout(C_, S_, G)
    consts = np.zeros((P, NCOLS), np.float32)
    consts[np.arange(P), o_g4 + np.arange(P) // Q] = 1.0 / HW_
    consts[np.arange(P) // Q, o_e4 + np.arange(P)] = 1.0
    consts[0:S_, o_gam : o_gam + C_] = gammas
    consts[0:S_, o_bet : o_bet + C_] = betas
    consts[0:S_, o_sw] = sw
    return consts


_CACHE = {}


def _get_nc():
    if "nc" not in _CACHE:
        _CACHE["nc"] = build_cin_program()
    return _CACHE["nc"]


def kernel(x, style_weights, gammas, betas, _trace=False):
    x = np.ascontiguousarray(np.asarray(x, dtype=np.float32))
    style_weights = np.ascontiguousarray(np.asarray(style_weights, dtype=np.float32))
    gammas = np.ascontiguousarray(np.asarray(gammas, dtype=np.float32))
    betas = np.ascontiguousarray(np.asarray(betas, dtype=np.float32))

    G = 32
    Q = P // G
    F = HW // Q
    nc = _get_nc()

    xr = x.reshape(B, C * Q, F)
    in_maps = [
        {
            "x": xr[i],
            "consts": make_consts(C, HW, S, G, gammas, betas, style_weights[i]),
        }
        for i in range(N_CORES)
    ]
    res = run_bass_kernel_spmd(
        nc, in_maps, core_ids=list(range(N_CORES)), trace=_trace
    )
    y = np.stack(
        [res.results[i]["y"].reshape(C, H, W) for i in range(N_CORES)], axis=0
    )
    if _trace:
        return y, res
    return y
